# revision 13
# baseline (speedup 1.0000x reference)
"""GQA attention prefill (B=4, S=2048, D=4096, 32 q-heads / 8 kv-heads, rotary,
causal) on 8 TRN2 NeuronCores.

Sharding: token-parallel ("zigzag" sequence split) — core c handles batch
c//2 and two 512-token chunks of its sequence: chunks {0,3} for even cores,
{1,2} for odd cores (balances the causal triangle). Each core computes Q and
K/V projections for ITS OWN tokens only; the K/V halves are exchanged with
the partner core (same batch) via two pairwise AllGather collectives, so the
K/V projection work is not duplicated. Attention and the output projection
run on each core's own tokens; outputs are disjoint token slices gathered on
host.

Precision: all four projections (Q/K/V/O) run as 3-term split-fp8 GEMMs in
DoubleRow perf mode (4x PE rate): x ~ (xh + xl), w ~ (wh + wl) with
power-of-2 pre-scales chosen to keep fp8e4m3 values in the normal range;
x@w ~ xh@wh + xh@wl + xl@wh accumulated in one PSUM group. The de-scale
folds into the eviction constants (rotary cos/sin tables and bias vectors)
at zero device cost. Attention scores/AV run f32r/bf16 exactly as before.

Device layout conventions:
  - activations for QK^T are kept transposed: [head_dim (partitions), tokens]
  - rotary pairs are de-interleaved (even dims -> partitions 0-63, odd ->
    64-127) via a host-side permutation of the qw/kw rows; scores are
    invariant to this shared permutation.
  - attention runs in "scores-transposed" orientation: ST[key, query] =
    kT.T @ qT, softmax over the partition (key) axis with the denominator
    computed by a ones-vector matmul; no max-subtraction (scores are O(1)).
  - DMA routing: large batched loads on nc.sync (HWDGE); high-count
    weight/output streams on nc.gpsimd (SWDGE, Pool engine).
"""

import numpy as np
import ml_dtypes

import concourse.bacc as bacc
import concourse.bass as bass
import concourse.tile as tile
from concourse import library_config, mybir
from concourse.bass_utils import run_bass_kernel_spmd

F32 = mybir.dt.float32
F32R = mybir.dt.float32r
BF16 = mybir.dt.bfloat16
FP8 = mybir.dt.float8e4
DR = mybir.MatmulPerfMode.DoubleRow
EXP = mybir.ActivationFunctionType.Exp
COPY = mybir.ActivationFunctionType.Copy
ADD = mybir.AluOpType.add
MULT = mybir.AluOpType.mult
SUB = mybir.AluOpType.subtract

B, S, D = 4, 2048, 4096
QH, KVH, HEAD = 32, 8, 128
P = 128
CH = 512                # token chunk (= query tile)
NCH = S // CH           # 4 chunks per sequence
ND = D // P             # 32 d-tiles
NKP = ND // 2           # 16 DoubleRow k-tile pairs
NCORES = 8
NKB = (8, 16)           # key-blocks per query slot (padded, uniform)
SCALE = 1.0 / np.sqrt(HEAD)
BF = ml_dtypes.bfloat16
E4 = ml_dtypes.float8_e4m3

SX = 32.0               # fp8 pre-scale: activations (input x)
SW = 1024.0             # fp8 pre-scale: projection weights
SO = 32.0               # fp8 pre-scale: attention output (o_attn)
INV_XW = 1.0 / (SX * SW)
INV_OW = 1.0 / (SO * SW)

_CACHE = {}


def _build():
    nc = bacc.Bacc("TRN2", target_bir_lowering=False, debug=False, num_devices=NCORES)

    # ---- per-core external inputs ----
    # own strips, hi/lo fp8: [hl, slot, dp, dt, t]
    own = nc.dram_tensor("own8", [2, 2, P, ND, CH], FP8, kind="ExternalInput")
    # weights, hi/lo fp8; [hl, head, dp, dt, fp] for q/k
    qwT = nc.dram_tensor("qwT8", [2, QH, P, ND, P], FP8, kind="ExternalInput")
    kwT = nc.dram_tensor("kwT8", [2, KVH, P, ND, P], FP8, kind="ExternalInput")
    # [hl, hs, kp, dp, 2, j]
    vwT = nc.dram_tensor("vwT8", [2, 2, NKP, P, 2, 512], FP8, kind="ExternalInput")
    # [hl, e, fpair, fp, 2, j]
    owT = nc.dram_tensor("owT8", [2, 8, NKP, P, 2, 512], FP8, kind="ExternalInput")
    cos_own = nc.dram_tensor("cos_own", [64, 2, CH], F32, kind="ExternalInput")
    sin_own = nc.dram_tensor("sin_own", [64, 2, CH], F32, kind="ExternalInput")
    qbT = nc.dram_tensor("qbT", [P, QH], F32, kind="ExternalInput")
    kbT = nc.dram_tensor("kbT", [P, KVH], F32, kind="ExternalInput")
    vb = nc.dram_tensor("vb", [KVH * HEAD], F32, kind="ExternalInput")
    ob = nc.dram_tensor("ob", [D], F32, kind="ExternalInput")
    masks = nc.dram_tensor("masks", [2, 8, P, CH], BF16, kind="ExternalInput")
    ones = nc.dram_tensor("ones", [P], BF16, kind="ExternalInput")

    # ---- internal DRAM ----
    qT_i = nc.dram_tensor("qT_i", [2, QH, P, CH], BF16)
    # own K/V halves + pairwise-gathered full tensors
    kT_own = nc.dram_tensor("kT_own", [KVH, P, 2, CH], BF16)
    kT_gat = nc.dram_tensor("kT_gat", [2, KVH, P, 2, CH], BF16)
    v_own = nc.dram_tensor("v_own", [2, 2, 4, P, 512], BF16)  # [hs, sl, b, kj, j]
    v_gat = nc.dram_tensor("v_gat", [2, 2, 2, 4, P, 512], BF16)
    oT_h = nc.dram_tensor("oT_h", [2, QH, P, CH], FP8)
    oT_l = nc.dram_tensor("oT_l", [2, QH, P, CH], FP8)

    out = nc.dram_tensor("out", [8, P, D], F32, kind="ExternalOutput")

    PAIRS = [[0, 1], [2, 3], [4, 5], [6, 7]]

    with tile.TileContext(nc) as tc:
        nc.gpsimd.load_library(library_config.lib)
        with (
            tc.tile_pool(name="const", bufs=1) as const,
            tc.tile_pool(name="ev", bufs=4) as evpool,
            tc.tile_pool(name="rt", bufs=4) as rtpool,
            tc.tile_pool(name="ps", bufs=8, space="PSUM") as pspool,
        ):
            kbT_e = const.tile([64, KVH], F32, tag="kbte")
            kbT_o = const.tile([64, KVH], F32, tag="kbto")
            nc.sync.dma_start(out=kbT_e[:], in_=kbT[0:64, :])
            nc.sync.dma_start(out=kbT_o[:], in_=kbT[64:P, :])
            ones_col = const.tile([P, 1], BF16, tag="oc")
            nc.sync.dma_start(out=ones_col[:], in_=ones.ap()[:, None])
            cos_own_sb = const.tile([64, 2, CH], F32, tag="coso")
            sin_own_sb = const.tile([64, 2, CH], F32, tag="sino")
            nc.sync.dma_start(out=cos_own_sb[:], in_=cos_own[:])
            nc.sync.dma_start(out=sin_own_sb[:], in_=sin_own[:])

            def rotary_evict(ps, dst, cos_ap, sin_ap, be, bo):
                """dst[0:64]=(pe+be)*cos-(po+bo)*sin; dst[64:128]=(pe+be)*sin+(po+bo)*cos

                cos/sin tables carry the 1/(SX*SW) de-scale; be/bo carry SX*SW.
                """
                pe, po = ps[0:64, :], ps[64:128, :]
                t1 = rtpool.tile([64, CH], F32, tag="rt", name="t1")
                t2 = rtpool.tile([64, CH], F32, tag="rt", name="t2")
                nc.vector.scalar_tensor_tensor(t1[:], pe, be, cos_ap, ADD, MULT)
                nc.vector.scalar_tensor_tensor(t2[:], po, bo, sin_ap, ADD, MULT)
                nc.vector.tensor_sub(dst[0:64, :], t1[:], t2[:])
                t3 = rtpool.tile([64, CH], F32, tag="rt", name="t3")
                t4 = rtpool.tile([64, CH], F32, tag="rt", name="t4")
                nc.vector.scalar_tensor_tensor(t3[:], pe, be, sin_ap, ADD, MULT)
                nc.vector.scalar_tensor_tensor(t4[:], po, bo, cos_ap, ADD, MULT)
                nc.vector.tensor_add(dst[64:128, :], t3[:], t4[:])

            # attention-phase pools entered first so later pools (strip/w)
            # can release in LIFO order right after attention slot 0
            kv_cm = tc.tile_pool(name="kvS", bufs=3)
            kvpool = kv_cm.__enter__()
            qt_cm = tc.tile_pool(name="qtS", bufs=2)
            qtpool = qt_cm.__enter__()
            pt_cm = tc.tile_pool(name="ptS", bufs=4)
            ptpool = pt_cm.__enter__()
            r_cm = tc.tile_pool(name="rS", bufs=2)
            rpool = r_cm.__enter__()

            # ---- own strips (hi/lo fp8), used by K/V and Q projections ----
            strip_cm = tc.tile_pool(name="strip", bufs=1)
            strip_pool = strip_cm.__enter__()
            st = {}
            for hl in range(2):
                for sl in range(2):
                    t = strip_pool.tile([P, ND, CH], FP8, tag=f"strip{hl}{sl}",
                                        name=f"st{hl}_{sl}")
                    nc.sync.dma_start(out=t[:], in_=own[hl, sl])
                    st[(hl, sl)] = t

            w_cm = tc.tile_pool(name="w", bufs=8)
            wpool = w_cm.__enter__()

            # 3-term schedule: (x_part, w_part) pairs, ll dropped
            TERMS = [(0, 0), (0, 1), (1, 0)]

            # ============ P1a: K projection over own chunks ============
            vb_cm = tc.tile_pool(name="p1c", bufs=1)
            p1c = vb_cm.__enter__()
            vb_sb = p1c.tile([P, KVH * HEAD], F32, tag="vb")
            nc.sync.dma_start(
                out=vb_sb[:], in_=vb.ap()[None, :].partition_broadcast(P)
            )
            for kv in range(KVH):
                wk = []
                for hl in range(2):
                    w = wpool.tile([P, ND, P], FP8, tag="w", name=f"kw{kv}_{hl}")
                    nc.gpsimd.dma_start(out=w[:], in_=kwT[hl, kv])
                    wk.append(w)
                for sl in range(2):
                    ps = pspool.tile([P, CH], F32, tag="ps", name="ps_k")
                    n3 = len(TERMS) * NKP
                    i = 0
                    for xp, wp in TERMS:
                        for kp in range(NKP):
                            nc.tensor.matmul(
                                ps[:],
                                lhsT=wk[wp][:, 2 * kp : 2 * kp + 2, :],
                                rhs=st[(xp, sl)][:, 2 * kp : 2 * kp + 2, :],
                                start=(i == 0), stop=(i == n3 - 1),
                                perf_mode=DR,
                            )
                            i += 1
                    krot = evpool.tile([P, CH], BF16, tag="ev", bufs=8, name="krot")
                    rotary_evict(
                        ps, krot,
                        cos_own_sb[:, sl, :], sin_own_sb[:, sl, :],
                        kbT_e[:, kv : kv + 1], kbT_o[:, kv : kv + 1],
                    )
                    nc.sync.dma_start(out=kT_own[kv, :, sl, :], in_=krot[:])

            # pairwise K exchange (overlaps with V-pass + Q-proj below)
            nc.gpsimd.collective_compute(
                "AllGather", mybir.AluOpType.bypass, replica_groups=PAIRS,
                ins=[kT_own[:]], outs=[kT_gat[:]],
            )

            # ============ P1b: V projection over own chunks ============
            with tc.tile_pool(name="wb", bufs=6) as wbpool:
                for hs in range(2):
                    psv = [
                        pspool.tile([P, 512], F32, tag="ps", name=f"psv{i}")
                        for i in range(8)
                    ]
                    for kp in range(NKP):
                        vw = []
                        for hl in range(2):
                            w = wbpool.tile([P, 2, 512], FP8, tag="wb",
                                            name=f"vw{hl}")
                            # act-dge: keeps these off the Pool SEQ, which
                            # stalls on the collectives' input waits
                            nc.scalar.dma_start(out=w[:], in_=vwT[hl, hs, kp])
                            vw.append(w)
                        for xp, wp in TERMS:
                            for sl in range(2):
                                for tt in range(4):
                                    nc.tensor.matmul(
                                        psv[sl * 4 + tt][:],
                                        lhsT=st[(xp, sl)][
                                            :, 2 * kp : 2 * kp + 2,
                                            tt * P : (tt + 1) * P,
                                        ],
                                        rhs=vw[wp][:],
                                        start=(kp == 0 and (xp, wp) == TERMS[0]),
                                        stop=(kp == NKP - 1 and (xp, wp) == TERMS[-1]),
                                        perf_mode=DR,
                                    )
                    for sl in range(2):
                        for tt in range(4):
                            vsb = evpool.tile([P, 512], BF16, tag="evb", name="vsb")
                            nc.vector.scalar_tensor_tensor(
                                vsb[:], psv[sl * 4 + tt][:], INV_XW,
                                vb_sb[:, hs * 512 : (hs + 1) * 512], MULT, ADD,
                            )
                            nc.sync.dma_start(out=v_own[hs, sl, tt], in_=vsb[:])

            vb_cm.__exit__(None, None, None)

            # ============ P0: Q projection + rotary -> qT_i ============
            # Slot 0 is emitted eagerly; slot 1 is a generator woven into
            # P3-slot0's key-block loop.
            p0c_cm = tc.tile_pool(name="p0c", bufs=1)
            p0c = p0c_cm.__enter__()
            qbT_e = p0c.tile([64, QH], F32, tag="qbte")
            qbT_o = p0c.tile([64, QH], F32, tag="qbto")
            nc.sync.dma_start(out=qbT_e[:], in_=qbT[0:64, :])
            nc.sync.dma_start(out=qbT_o[:], in_=qbT[64:P, :])

            def p0_heads(sl, group):
                """Emit Q-proj for one slot; yield after each `group` matmuls."""
                for h in range(QH):
                    wq = []
                    for hl in range(2):
                        w = wpool.tile([P, ND, P], FP8, tag="w",
                                       name=f"qw{sl}_{h}_{hl}")
                        eng = nc.gpsimd if hl == 0 else nc.sync
                        eng.dma_start(out=w[:], in_=qwT[hl, h])
                        wq.append(w)
                    ps = pspool.tile([P, CH], F32, tag="ps", name="ps_q")
                    n3 = len(TERMS) * NKP
                    i = 0
                    for xp, wp in TERMS:
                        for kp in range(NKP):
                            nc.tensor.matmul(
                                ps[:],
                                lhsT=wq[wp][:, 2 * kp : 2 * kp + 2, :],
                                rhs=st[(xp, sl)][:, 2 * kp : 2 * kp + 2, :],
                                start=(i == 0), stop=(i == n3 - 1),
                                perf_mode=DR,
                            )
                            i += 1
                            if i % group == 0:
                                yield
                    qrot = evpool.tile([P, CH], BF16, tag="ev", bufs=8, name="qrot")
                    rotary_evict(
                        ps, qrot,
                        cos_own_sb[:, sl, :], sin_own_sb[:, sl, :],
                        qbT_e[:, h : h + 1], qbT_o[:, h : h + 1],
                    )
                    nc.sync.dma_start(out=qT_i[sl, h], in_=qrot[:])

            for _ in p0_heads(0, ND):
                pass

            # pairwise V exchange; emitted after Q-slot0 so the qw weight
            # loads are not queued behind this collective's input wait on
            # the Pool SEQ (they prefetch during the V-pass instead)
            nc.gpsimd.collective_compute(
                "AllGather", mybir.AluOpType.bypass, replica_groups=PAIRS,
                ins=[v_own[:]], outs=[v_gat[:]],
            )

            def p4_half(hf, yield_every, otr_h, otr_l, wb4pool, ob_sb):
                """Emit o-proj for token-slot half `hf` (ttiles 4hf..4hf+3)."""
                for hq in range(0, QH, 8):
                    nc.sync.dma_start(
                        out=otr_h[:, hq : hq + 8, :],
                        in_=oT_h[hf, hq : hq + 8].rearrange("h p t -> p h t"),
                    )
                    nc.sync.dma_start(
                        out=otr_l[:, hq : hq + 8, :],
                        in_=oT_l[hf, hq : hq + 8].rearrange("h p t -> p h t"),
                    )
                otr = (otr_h, otr_l)
                for e in range(8):
                    ps4 = [
                        pspool.tile([P, 512], F32, tag="ps", name=f"ps4_{i}")
                        for i in range(4)
                    ]
                    cnt = 0
                    n3 = len(TERMS) * NKP
                    for fq in range(NKP // 4):
                        ow = []
                        for hl in range(2):
                            w = wb4pool.tile([P, 4, 2, 512], FP8, tag="wb4",
                                             name=f"ow{hl}")
                            nc.gpsimd.dma_start(
                                out=w[:],
                                in_=owT[hl, e, 4 * fq : 4 * fq + 4].rearrange(
                                    "q p two j -> p q two j"
                                ),
                            )
                            ow.append(w)
                        for df in range(4):
                            fp = 4 * fq + df
                            for xp, wp in TERMS:
                                for tsub in range(4):
                                    nc.tensor.matmul(
                                        ps4[tsub][:],
                                        lhsT=otr[xp][
                                            :, 2 * fp : 2 * fp + 2,
                                            tsub * P : (tsub + 1) * P,
                                        ],
                                        rhs=ow[wp][:, df, :, :],
                                        start=(fp == 0 and (xp, wp) == TERMS[0]),
                                        stop=(fp == NKP - 1
                                              and (xp, wp) == TERMS[-1]),
                                        perf_mode=DR,
                                    )
                                    cnt += 1
                                    if cnt % yield_every == 0:
                                        yield
                    for tsub in range(4):
                        osb = evpool.tile([P, 512], F32, tag="ev4", name="osb4")
                        nc.vector.scalar_tensor_tensor(
                            osb[:], ps4[tsub][:], INV_OW,
                            ob_sb[:, e * 512 : (e + 1) * 512], MULT, ADD,
                        )
                        nc.sync.dma_start(
                            out=out[hf * 4 + tsub, :, e * 512 : (e + 1) * 512],
                            in_=osb[:],
                        )

            def attn_slot(sl, feeder):
                n_kb = NKB[sl]
                with (
                    tc.tile_pool(name=f"mask{sl}", bufs=1) as mpool,
                    tc.tile_pool(name=f"v4{sl}", bufs=1) as v4pool,
                ):
                    msk = mpool.tile([P, 8, CH], BF16, tag="mask", name="msk")
                    nc.scalar.dma_start(
                        out=msk[:], in_=masks[sl].rearrange("m k q -> k m q")
                    )
                    for hs in range(2):
                        v4 = v4pool.tile([P, n_kb, 512], BF16, tag="v4", name="v4")
                        nc.scalar.dma_start(
                            out=v4[:, 0:4, :],
                            in_=v_gat[0, hs, 0].rearrange("b p j -> p b j"),
                        )
                        nc.scalar.dma_start(
                            out=v4[:, 4:8, :],
                            in_=v_gat[1, hs, 0].rearrange("b p j -> p b j"),
                        )
                        if sl == 1:
                            nc.scalar.dma_start(
                                out=v4[:, 8:12, :],
                                in_=v_gat[1, hs, 1].rearrange("b p j -> p b j"),
                            )
                            nc.scalar.dma_start(
                                out=v4[:, 12:16, :],
                                in_=v_gat[0, hs, 1].rearrange("b p j -> p b j"),
                            )
                        for j in range(4):
                            kv = 4 * hs + j
                            kt = kvpool.tile([P, n_kb * P], BF16, tag="kt", name="kt")
                            nc.scalar.dma_start(
                                out=kt[:, 0 : 2 * CH],
                                in_=kT_gat[:, kv, :, 0, :].rearrange(
                                    "c p t -> p c t"
                                ),
                            )
                            if sl == 1:
                                nc.scalar.dma_start(
                                    out=kt[:, 2 * CH : 3 * CH],
                                    in_=kT_gat[1, kv, :, 1, :],
                                )
                                nc.scalar.dma_start(
                                    out=kt[:, 3 * CH : 4 * CH],
                                    in_=kT_gat[0, kv, :, 1, :],
                                )
                            qt4 = qtpool.tile([P, 4, CH], BF16, tag="qt", name="qt4")
                            nc.scalar.dma_start(
                                out=qt4[:],
                                in_=qT_i[sl, kv :: KVH].rearrange("g p t -> p g t"),
                            )
                            for g in range(4):
                                h = kv + KVH * g
                                oT_ps = pspool.tile([P, CH], F32, tag="ps", name="oT_ps")
                                sums_ps = pspool.tile([P, CH], F32, tag="ps", name="sums_ps")
                                for kb in range(n_kb):
                                    st_ps = pspool.tile([P, CH], F32, tag="ps", name="st_ps")
                                    nc.tensor.matmul(
                                        st_ps[:],
                                        lhsT=kt[:, kb * P : (kb + 1) * P],
                                        rhs=qt4[:, g, :], start=True, stop=True,
                                    )
                                    pt = ptpool.tile([P, CH], BF16, tag="pt", name="pt")
                                    nc.scalar.activation(pt[:], st_ps[:], EXP, scale=SCALE)
                                    if sl == 0 or kb >= 8:
                                        mi = kb if sl == 0 else kb - 8
                                        nc.vector.tensor_mul(pt[:], pt[:], msk[:, mi, :])
                                    if kb % 2 == 0:
                                        pt_prev = pt
                                    else:
                                        pp = ptpool.tile([P, CH], BF16, tag="ptp", name="pp")
                                        nc.vector.tensor_add(pp[:], pt_prev[:], pt[:])
                                        if kb % 4 == 1:
                                            pp_prev = pp
                                        else:
                                            pq = ptpool.tile([P, CH], BF16, tag="ptq", name="pq")
                                            nc.vector.tensor_add(pq[:], pp_prev[:], pp[:])
                                            nc.tensor.matmul(
                                                sums_ps[0:1, :], lhsT=ones_col[:], rhs=pq[:],
                                                start=(kb == 3), stop=(kb == n_kb - 1),
                                            )
                                    nc.tensor.matmul(
                                        oT_ps[:],
                                        lhsT=v4[:, kb, j * P : (j + 1) * P],
                                        rhs=pt[:],
                                        start=(kb == 0), stop=(kb == n_kb - 1),
                                    )
                                    if feeder is not None:
                                        next(feeder, None)
                                rsb = rpool.tile([1, CH], F32R, tag="r", name="rsb")
                                with nc.allow_low_precision(reason="f32r softmax denom"):
                                    nc.vector.reciprocal(rsb[:], sums_ps[0:1, :])
                                rb_bc = ptpool.tile([P, CH], F32R, tag="ptr", name="rb_bc")
                                nc.gpsimd.partition_broadcast(rb_bc[:], rsb[:])
                                # o32s = SO * normalized o (ones carry 1/SO)
                                o32s = evpool.tile([P, CH], F32, tag="evb", name="o32s")
                                nc.vector.tensor_mul(o32s[:], oT_ps[:], rb_bc[:])
                                oh = evpool.tile([P, CH], FP8, tag="evh", name="oh")
                                nc.scalar.activation(oh[:], o32s[:], COPY)
                                ol = evpool.tile([P, CH], FP8, tag="evl", name="ol")
                                nc.gpsimd.tensor_sub(ol[:], o32s[:], oh[:])
                                nc.sync.dma_start(out=oT_h[sl, h], in_=oh[:])
                                nc.sync.dma_start(out=oT_l[sl, h], in_=ol[:])
                    if feeder is not None:
                        for _ in feeder:
                            pass

            # ==== P3 slot 0 woven with P0 slot 1 ====
            attn_slot(0, p0_heads(1, 4))
            p0c_cm.__exit__(None, None, None)
            w_cm.__exit__(None, None, None)
            strip_cm.__exit__(None, None, None)

            # ==== P3 slot 1 woven with P4 half 0; then P4 half 1 ====
            with tc.tile_pool(name="obp", bufs=1) as obp:
                ob_sb = obp.tile([P, D], F32, tag="ob")
                nc.sync.dma_start(
                    out=ob_sb[:], in_=ob.ap()[None, :].partition_broadcast(P)
                )
                with (
                    tc.tile_pool(name="p4a", bufs=1) as p4a,
                    tc.tile_pool(name="wb4a", bufs=4) as wb4a,
                ):
                    otr0h = p4a.tile([P, QH, CH], FP8, tag="ot0h")
                    otr0l = p4a.tile([P, QH, CH], FP8, tag="ot0l")
                    attn_slot(1, p4_half(0, 3, otr0h, otr0l, wb4a, ob_sb))
                with (
                    tc.tile_pool(name="p4b", bufs=1) as p4b,
                    tc.tile_pool(name="wb4b", bufs=4) as wb4b,
                ):
                    otr1h = p4b.tile([P, QH, CH], FP8, tag="ot1h")
                    otr1l = p4b.tile([P, QH, CH], FP8, tag="ot1l")
                    for _ in p4_half(1, 1 << 30, otr1h, otr1l, wb4b, ob_sb):
                        pass
            r_cm.__exit__(None, None, None)
            pt_cm.__exit__(None, None, None)
            qt_cm.__exit__(None, None, None)
            kv_cm.__exit__(None, None, None)

    nc.compile()
    return nc


def _get_nc():
    if "nc" not in _CACHE:
        _CACHE["nc"] = _build()
    return _CACHE["nc"]


_PERM = np.concatenate([np.arange(0, P, 2), np.arange(1, P, 2)])


def _split8(a, s):
    """Return (hi, lo) fp8 arrays of a*s."""
    a = np.clip(a * s, -240.0, 240.0)
    hi = a.astype(E4)
    lo = (a - hi.astype(np.float32)).astype(E4)
    return hi, lo


def _prep_shared(qw_w, qw_b, kw_w, kw_b, vw_w, vw_b, ow_w, ow_b, fc, fs):
    f32 = np.float32
    c = np.ascontiguousarray

    # [h, dp, dt, fp] = w[h*128 + perm[fp], dt*128 + dp]
    qq = qw_w.reshape(QH, P, D)[:, _PERM, :]                      # [h, fp, d]
    qwT = qq.reshape(QH, P, ND, P).transpose(0, 3, 2, 1)
    qwT8 = np.stack(_split8(qwT, SW))                             # [2, h, dp, dt, fp]
    kk = kw_w.reshape(KVH, P, D)[:, _PERM, :]
    kwT = kk.reshape(KVH, P, ND, P).transpose(0, 3, 2, 1)
    kwT8 = np.stack(_split8(kwT, SW))
    # [hs, kp, dp, 2, j] = vw[hs*512 + j, (2kp+two)*128 + dp]
    vwT = vw_w.reshape(2, 512, NKP, 2, P).transpose(0, 2, 4, 3, 1)
    vwT8 = np.stack(_split8(vwT, SW))
    # [e, fpair, fp, 2, j] = ow[e*512 + j, (2fpair+two)*128 + fp]
    owT = ow_w.reshape(8, 512, NKP, 2, P).transpose(0, 2, 4, 3, 1)
    owT8 = np.stack(_split8(owT, SW))
    cos_all = c(fc.T.astype(f32)) * np.float32(INV_XW)  # [64, S], carries de-scale
    sin_all = c(fs.T.astype(f32)) * np.float32(INV_XW)
    qbT = c(qw_b.reshape(QH, P)[:, _PERM].T.astype(f32)) * np.float32(SX * SW)
    kbT = c(kw_b.reshape(KVH, P)[:, _PERM].T.astype(f32)) * np.float32(SX * SW)
    return dict(
        qwT8=c(qwT8), kwT8=c(kwT8), vwT8=c(vwT8), owT8=c(owT8),
        cos_all=cos_all, sin_all=sin_all, qbT=qbT, kbT=kbT,
        vb=c(vw_b.astype(f32)), ob=c(ow_b.astype(f32)),
    )


def _masks_for(chunks):
    m = np.zeros((2, 8, P, CH), BF)
    kp = np.arange(P)[:, None]
    qi = np.arange(CH)[None, :]
    for sl in range(2):
        q0 = chunks[sl] * CH
        for mi in range(8):
            kb = mi if sl == 0 else mi + 8
            m[sl, mi] = (kb * P + kp <= q0 + qi).astype(BF)
    return m


def _core_chunks(core):
    b, par = core // 2, core % 2
    return b, ((0, 3) if par == 0 else (1, 2))


def _make_in_maps(inputs):
    """inputs: dict with the reference's setup_inputs() keys (numpy)."""
    g = lambda k: np.asarray(inputs[k])
    shared = _prep_shared(
        g("qw_w"), g("qw_b"), g("kw_w"), g("kw_b"), g("vw_w"), g("vw_b"),
        g("ow_w"), g("ow_b"), g("freqs_cos"), g("freqs_sin"),
    )
    input = g("input")
    in_maps = []
    for core in range(NCORES):
        b, chunks = _core_chunks(core)
        x = input[b].astype(np.float32)  # [S, D]
        # [s, dp, dt, t] = x[s*512 + t, dt*128 + dp]
        strips = x.reshape(NCH, CH, ND, P).transpose(0, 3, 2, 1)
        own_f = strips[list(chunks)]                       # [2, dp, dt, t]
        own8 = np.stack(_split8(own_f, SX))                # [2(hl), 2, dp, dt, t]
        cos_own = np.ascontiguousarray(
            np.stack([shared["cos_all"][:, c * CH : (c + 1) * CH] for c in chunks], 1)
        )
        sin_own = np.ascontiguousarray(
            np.stack([shared["sin_all"][:, c * CH : (c + 1) * CH] for c in chunks], 1)
        )
        m = {k: v for k, v in shared.items() if k not in ("cos_all", "sin_all")}
        m.update(
            ones=np.full(P, 1.0 / SO, BF),
            own8=np.ascontiguousarray(own8),
            cos_own=cos_own, sin_own=sin_own, masks=_masks_for(chunks),
        )
        in_maps.append(m)
    return in_maps


def kernel(input, freqs_cos, freqs_sin, qw_w, qw_b, kw_w, kw_b, vw_w, vw_b,
           ow_w, ow_b, start_pos):
    in_maps = _make_in_maps(dict(
        input=input, freqs_cos=freqs_cos, freqs_sin=freqs_sin,
        qw_w=qw_w, qw_b=qw_b, kw_w=kw_w, kw_b=kw_b, vw_w=vw_w, vw_b=vw_b,
        ow_w=ow_w, ow_b=ow_b,
    ))
    nc = _get_nc()
    res = run_bass_kernel_spmd(nc, in_maps, list(range(NCORES)))

    out = np.empty((B, S, D), np.float32)
    for core in range(NCORES):
        b, chunks = _core_chunks(core)
        r = res.results[core]["out"].reshape(2, CH, D)
        for sl in range(2):
            c0 = chunks[sl] * CH
            out[b, c0 : c0 + CH, :] = r[sl]
    return out


# revision 16
# speedup vs baseline: 1.0354x; 1.0354x over previous
"""GQA attention prefill (B=4, S=2048, D=4096, 32 q-heads / 8 kv-heads, rotary,
causal) on 8 TRN2 NeuronCores.

Sharding: token-parallel ("zigzag" sequence split) — core c handles batch
c//2 and two 512-token chunks of its sequence: chunks {0,3} for even cores,
{1,2} for odd cores (balances the causal triangle). Each core computes Q and
K/V projections for ITS OWN tokens only; the K/V halves are exchanged with
the partner core (same batch) via two pairwise AllGather collectives, so the
K/V projection work is not duplicated. Attention and the output projection
run on each core's own tokens; outputs are disjoint token slices gathered on
host.

Precision: all four projections (Q/K/V/O) run as 3-term split-fp8 GEMMs in
DoubleRow perf mode (4x PE rate): x ~ (xh + xl), w ~ (wh + wl) with
power-of-2 pre-scales chosen to keep fp8e4m3 values in the normal range;
x@w ~ xh@wh + xh@wl + xl@wh accumulated in one PSUM group. The de-scale
folds into the eviction constants (rotary cos/sin tables and bias vectors)
at zero device cost. Attention scores/AV run f32r/bf16 exactly as before.

Device layout conventions:
  - activations for QK^T are kept transposed: [head_dim (partitions), tokens]
  - rotary pairs are de-interleaved (even dims -> partitions 0-63, odd ->
    64-127) via a host-side permutation of the qw/kw rows; scores are
    invariant to this shared permutation.
  - attention runs in "scores-transposed" orientation: ST[key, query] =
    kT.T @ qT, softmax over the partition (key) axis with the denominator
    computed by a ones-vector matmul; no max-subtraction (scores are O(1)).
  - DMA routing: large batched loads on nc.sync (HWDGE); high-count
    weight/output streams on nc.gpsimd (SWDGE, Pool engine).
"""

import numpy as np
import ml_dtypes

import concourse.bacc as bacc
import concourse.bass as bass
import concourse.tile as tile
from concourse import library_config, mybir
from concourse.bass_utils import run_bass_kernel_spmd

F32 = mybir.dt.float32
F32R = mybir.dt.float32r
BF16 = mybir.dt.bfloat16
FP8 = mybir.dt.float8e4
DR = mybir.MatmulPerfMode.DoubleRow
EXP = mybir.ActivationFunctionType.Exp
COPY = mybir.ActivationFunctionType.Copy
ADD = mybir.AluOpType.add
MULT = mybir.AluOpType.mult
SUB = mybir.AluOpType.subtract

B, S, D = 4, 2048, 4096
QH, KVH, HEAD = 32, 8, 128
P = 128
CH = 512                # token chunk (= query tile)
NCH = S // CH           # 4 chunks per sequence
ND = D // P             # 32 d-tiles
NKP = ND // 2           # 16 DoubleRow k-tile pairs
NCORES = 8
NKB = (8, 16)           # key-blocks per query slot (padded, uniform)
SCALE = 1.0 / np.sqrt(HEAD)
BF = ml_dtypes.bfloat16
E4 = ml_dtypes.float8_e4m3

SX = 32.0               # fp8 pre-scale: activations (input x)
SW = 1024.0             # fp8 pre-scale: projection weights
SO = 32.0               # fp8 pre-scale: attention output (o_attn)
INV_XW = 1.0 / (SX * SW)
INV_OW = 1.0 / (SO * SW)

_CACHE = {}


def _build():
    nc = bacc.Bacc("TRN2", target_bir_lowering=False, debug=False, num_devices=NCORES)

    # ---- per-core external inputs ----
    # own strips, hi/lo fp8: [hl, slot, dp, dt, t]
    own = nc.dram_tensor("own8", [2, 2, P, ND, CH], FP8, kind="ExternalInput")
    # weights, hi/lo fp8; [hl, head, dp, dt, fp] for q/k
    qwT = nc.dram_tensor("qwT8", [2, QH, P, ND, P], FP8, kind="ExternalInput")
    kwT = nc.dram_tensor("kwT8", [2, KVH, P, ND, P], FP8, kind="ExternalInput")
    # [hl, hs, kp, dp, 2, j]
    vwT = nc.dram_tensor("vwT8", [2, 2, NKP, P, 2, 512], FP8, kind="ExternalInput")
    # [hl, e, fpair, fp, 2, j]
    owT = nc.dram_tensor("owT8", [2, 8, NKP, P, 2, 512], FP8, kind="ExternalInput")
    cos_own = nc.dram_tensor("cos_own", [64, 2, CH], F32, kind="ExternalInput")
    sin_own = nc.dram_tensor("sin_own", [64, 2, CH], F32, kind="ExternalInput")
    qbT = nc.dram_tensor("qbT", [P, QH], F32, kind="ExternalInput")
    kbT = nc.dram_tensor("kbT", [P, KVH], F32, kind="ExternalInput")
    vb = nc.dram_tensor("vb", [KVH * HEAD], F32, kind="ExternalInput")
    ob = nc.dram_tensor("ob", [D], F32, kind="ExternalInput")
    masks = nc.dram_tensor("masks", [2, 8, P, CH], BF16, kind="ExternalInput")
    ones = nc.dram_tensor("ones", [P], BF16, kind="ExternalInput")

    # ---- internal DRAM ----
    qT_i = nc.dram_tensor("qT_i", [2, QH, P, CH], BF16)
    # own K/V halves + pairwise-gathered full tensors
    # per-slot tensors (separate handles so the slot-0 collective's read does
    # not create a false whole-tensor WAR dependency against slot-1 writes)
    kT_own = [nc.dram_tensor(f"kT_own{s}", [KVH, P, CH], BF16) for s in range(2)]
    kT_gat = [nc.dram_tensor(f"kT_gat{s}", [2, KVH, P, CH], BF16) for s in range(2)]
    v_own = [nc.dram_tensor(f"v_own{s}", [2, 4, P, 512], BF16) for s in range(2)]
    v_gat = [nc.dram_tensor(f"v_gat{s}", [2, 2, 4, P, 512], BF16) for s in range(2)]
    oT_h = nc.dram_tensor("oT_h", [2, QH, P, CH], FP8)
    oT_l = nc.dram_tensor("oT_l", [2, QH, P, CH], FP8)

    out = nc.dram_tensor("out", [8, P, D], F32, kind="ExternalOutput")

    PAIRS = [[0, 1], [2, 3], [4, 5], [6, 7]]

    with tile.TileContext(nc) as tc:
        nc.gpsimd.load_library(library_config.lib)
        with (
            tc.tile_pool(name="const", bufs=1) as const,
            tc.tile_pool(name="ev", bufs=4) as evpool,
            tc.tile_pool(name="rt", bufs=4) as rtpool,
            tc.tile_pool(name="ps", bufs=8, space="PSUM") as pspool,
        ):
            kbT_e = const.tile([64, KVH], F32, tag="kbte")
            kbT_o = const.tile([64, KVH], F32, tag="kbto")
            nc.sync.dma_start(out=kbT_e[:], in_=kbT[0:64, :])
            nc.sync.dma_start(out=kbT_o[:], in_=kbT[64:P, :])
            ones_col = const.tile([P, 1], BF16, tag="oc")
            nc.sync.dma_start(out=ones_col[:], in_=ones.ap()[:, None])
            cos_own_sb = const.tile([64, 2, CH], F32, tag="coso")
            sin_own_sb = const.tile([64, 2, CH], F32, tag="sino")
            nc.sync.dma_start(out=cos_own_sb[:], in_=cos_own[:])
            nc.sync.dma_start(out=sin_own_sb[:], in_=sin_own[:])

            def rotary_evict(ps, dst, cos_ap, sin_ap, be, bo):
                """dst[0:64]=(pe+be)*cos-(po+bo)*sin; dst[64:128]=(pe+be)*sin+(po+bo)*cos

                cos/sin tables carry the 1/(SX*SW) de-scale; be/bo carry SX*SW.
                """
                pe, po = ps[0:64, :], ps[64:128, :]
                t1 = rtpool.tile([64, CH], F32, tag="rt", name="t1")
                t2 = rtpool.tile([64, CH], F32, tag="rt", name="t2")
                nc.vector.scalar_tensor_tensor(t1[:], pe, be, cos_ap, ADD, MULT)
                nc.vector.scalar_tensor_tensor(t2[:], po, bo, sin_ap, ADD, MULT)
                nc.vector.tensor_sub(dst[0:64, :], t1[:], t2[:])
                t3 = rtpool.tile([64, CH], F32, tag="rt", name="t3")
                t4 = rtpool.tile([64, CH], F32, tag="rt", name="t4")
                nc.vector.scalar_tensor_tensor(t3[:], pe, be, sin_ap, ADD, MULT)
                nc.vector.scalar_tensor_tensor(t4[:], po, bo, cos_ap, ADD, MULT)
                nc.vector.tensor_add(dst[64:128, :], t3[:], t4[:])

            # attention-phase pools entered first so later pools (strip/w)
            # can release in LIFO order right after attention slot 0
            kv_cm = tc.tile_pool(name="kvS", bufs=3)
            kvpool = kv_cm.__enter__()
            qt_cm = tc.tile_pool(name="qtS", bufs=2)
            qtpool = qt_cm.__enter__()
            pt_cm = tc.tile_pool(name="ptS", bufs=4)
            ptpool = pt_cm.__enter__()
            r_cm = tc.tile_pool(name="rS", bufs=2)
            rpool = r_cm.__enter__()

            # ---- own strips (hi/lo fp8), used by K/V and Q projections ----
            strip_cm = tc.tile_pool(name="strip", bufs=1)
            strip_pool = strip_cm.__enter__()
            st = {}
            for sl in range(2):
                for hl in range(2):
                    t = strip_pool.tile([P, ND, CH], FP8, tag=f"strip{hl}{sl}",
                                        name=f"st{hl}_{sl}")
                    nc.sync.dma_start(out=t[:, 0:8, :], in_=own[hl, sl, :, 0:8])
                    nc.sync.dma_start(out=t[:, 8:ND, :], in_=own[hl, sl, :, 8:ND])
                    st[(hl, sl)] = t

            w_cm = tc.tile_pool(name="w", bufs=8)
            wpool = w_cm.__enter__()

            # 3-term schedule: (x_part, w_part) pairs, ll dropped
            TERMS = [(0, 0), (0, 1), (1, 0)]

            # ============ P1a: K projection over own chunks ============
            vb_cm = tc.tile_pool(name="p1c", bufs=1)
            p1c = vb_cm.__enter__()
            vb_sb = p1c.tile([P, KVH * HEAD], F32, tag="vb")
            nc.sync.dma_start(
                out=vb_sb[:], in_=vb.ap()[None, :].partition_broadcast(P)
            )
            for sl in range(2):
                for kv in range(KVH):
                    wk = []
                    for hl in range(2):
                        w = wpool.tile([P, ND, P], FP8, tag="w",
                                       name=f"kw{sl}_{kv}_{hl}")
                        nc.gpsimd.dma_start(out=w[:], in_=kwT[hl, kv])
                        wk.append(w)
                    ps = pspool.tile([P, CH], F32, tag="ps", name="ps_k")
                    n3 = len(TERMS) * NKP
                    i = 0
                    for xp, wp in TERMS:
                        for kp in range(NKP):
                            nc.tensor.matmul(
                                ps[:],
                                lhsT=wk[wp][:, 2 * kp : 2 * kp + 2, :],
                                rhs=st[(xp, sl)][:, 2 * kp : 2 * kp + 2, :],
                                start=(i == 0), stop=(i == n3 - 1),
                                perf_mode=DR,
                            )
                            i += 1
                    krot = evpool.tile([P, CH], BF16, tag="ev", bufs=8, name="krot")
                    rotary_evict(
                        ps, krot,
                        cos_own_sb[:, sl, :], sin_own_sb[:, sl, :],
                        kbT_e[:, kv : kv + 1], kbT_o[:, kv : kv + 1],
                    )
                    nc.sync.dma_start(out=kT_own[sl][kv], in_=krot[:])
                if sl == 0:
                    # pairwise K exchange for slot-0 chunks: lands well
                    # before attention slot 0 needs it
                    nc.gpsimd.collective_compute(
                        "AllGather", mybir.AluOpType.bypass, replica_groups=PAIRS,
                        ins=[kT_own[0][:]], outs=[kT_gat[0][:]],
                    )

            # ============ P1b: V projection over own chunks ============
            with tc.tile_pool(name="wb", bufs=6) as wbpool:
                for sl in range(2):
                    for hs in range(2):
                        psv = [
                            pspool.tile([P, 512], F32, tag="ps", name=f"psv{i}")
                            for i in range(4)
                        ]
                        for kp in range(NKP):
                            vw = []
                            for hl in range(2):
                                w = wbpool.tile([P, 2, 512], FP8, tag="wb",
                                                name=f"vw{hl}")
                                # sync queue: keeps these off the Pool SEQ
                                # (collective input waits) and off Act (the
                                # scheduler hoists collective-dependent attn
                                # loads there, which would stall these)
                                nc.sync.dma_start(out=w[:], in_=vwT[hl, hs, kp])
                                vw.append(w)
                            for xp, wp in TERMS:
                                for tt in range(4):
                                    nc.tensor.matmul(
                                        psv[tt][:],
                                        lhsT=st[(xp, sl)][
                                            :, 2 * kp : 2 * kp + 2,
                                            tt * P : (tt + 1) * P,
                                        ],
                                        rhs=vw[wp][:],
                                        start=(kp == 0 and (xp, wp) == TERMS[0]),
                                        stop=(kp == NKP - 1 and (xp, wp) == TERMS[-1]),
                                        perf_mode=DR,
                                    )
                        for tt in range(4):
                            vsb = evpool.tile([P, 512], BF16, tag="evb", name="vsb")
                            nc.vector.scalar_tensor_tensor(
                                vsb[:], psv[tt][:], INV_XW,
                                vb_sb[:, hs * 512 : (hs + 1) * 512], MULT, ADD,
                            )
                            nc.sync.dma_start(out=v_own[sl][hs, tt], in_=vsb[:])
                    # pairwise V exchange for this slot's chunks; the
                    # slot-1 K exchange is sequenced after the slot-0 V one
                    # (attention needs V0 ~500us before K1)
                    nc.gpsimd.collective_compute(
                        "AllGather", mybir.AluOpType.bypass, replica_groups=PAIRS,
                        ins=[v_own[sl][:]], outs=[v_gat[sl][:]],
                    )
                    if sl == 0:
                        nc.gpsimd.collective_compute(
                            "AllGather", mybir.AluOpType.bypass,
                            replica_groups=PAIRS,
                            ins=[kT_own[1][:]], outs=[kT_gat[1][:]],
                        )

            vb_cm.__exit__(None, None, None)

            # ============ P0: Q projection + rotary -> qT_i ============
            # Slot 0 is emitted eagerly; slot 1 is a generator woven into
            # P3-slot0's key-block loop.
            p0c_cm = tc.tile_pool(name="p0c", bufs=1)
            p0c = p0c_cm.__enter__()
            qbT_e = p0c.tile([64, QH], F32, tag="qbte")
            qbT_o = p0c.tile([64, QH], F32, tag="qbto")
            nc.sync.dma_start(out=qbT_e[:], in_=qbT[0:64, :])
            nc.sync.dma_start(out=qbT_o[:], in_=qbT[64:P, :])

            def p0_heads(sl, group):
                """Emit Q-proj for one slot; yield after each `group` matmuls."""
                for h in range(QH):
                    wq = []
                    for hl in range(2):
                        w = wpool.tile([P, ND, P], FP8, tag="w",
                                       name=f"qw{sl}_{h}_{hl}")
                        eng = nc.gpsimd if hl == 0 else nc.sync
                        eng.dma_start(out=w[:], in_=qwT[hl, h])
                        wq.append(w)
                    ps = pspool.tile([P, CH], F32, tag="ps", name="ps_q")
                    n3 = len(TERMS) * NKP
                    i = 0
                    for xp, wp in TERMS:
                        for kp in range(NKP):
                            nc.tensor.matmul(
                                ps[:],
                                lhsT=wq[wp][:, 2 * kp : 2 * kp + 2, :],
                                rhs=st[(xp, sl)][:, 2 * kp : 2 * kp + 2, :],
                                start=(i == 0), stop=(i == n3 - 1),
                                perf_mode=DR,
                            )
                            i += 1
                            if i % group == 0:
                                yield
                    qrot = evpool.tile([P, CH], BF16, tag="ev", bufs=8, name="qrot")
                    rotary_evict(
                        ps, qrot,
                        cos_own_sb[:, sl, :], sin_own_sb[:, sl, :],
                        qbT_e[:, h : h + 1], qbT_o[:, h : h + 1],
                    )
                    nc.sync.dma_start(out=qT_i[sl, h], in_=qrot[:])

            for _ in p0_heads(0, ND):
                pass


            def p4_half(hf, yield_every, otr_h, otr_l, wb4pool, ob_sb):
                """Emit o-proj for token-slot half `hf` (ttiles 4hf..4hf+3)."""
                for hq in range(0, QH, 8):
                    nc.sync.dma_start(
                        out=otr_h[:, hq : hq + 8, :],
                        in_=oT_h[hf, hq : hq + 8].rearrange("h p t -> p h t"),
                    )
                    nc.sync.dma_start(
                        out=otr_l[:, hq : hq + 8, :],
                        in_=oT_l[hf, hq : hq + 8].rearrange("h p t -> p h t"),
                    )
                otr = (otr_h, otr_l)
                for e in range(8):
                    ps4 = [
                        pspool.tile([P, 512], F32, tag="ps", name=f"ps4_{i}")
                        for i in range(4)
                    ]
                    cnt = 0
                    n3 = len(TERMS) * NKP
                    for fq in range(NKP // 4):
                        ow = []
                        for hl in range(2):
                            w = wb4pool.tile([P, 4, 2, 512], FP8, tag="wb4",
                                             name=f"ow{hl}")
                            nc.gpsimd.dma_start(
                                out=w[:],
                                in_=owT[hl, e, 4 * fq : 4 * fq + 4].rearrange(
                                    "q p two j -> p q two j"
                                ),
                            )
                            ow.append(w)
                        for df in range(4):
                            fp = 4 * fq + df
                            for xp, wp in TERMS:
                                for tsub in range(4):
                                    nc.tensor.matmul(
                                        ps4[tsub][:],
                                        lhsT=otr[xp][
                                            :, 2 * fp : 2 * fp + 2,
                                            tsub * P : (tsub + 1) * P,
                                        ],
                                        rhs=ow[wp][:, df, :, :],
                                        start=(fp == 0 and (xp, wp) == TERMS[0]),
                                        stop=(fp == NKP - 1
                                              and (xp, wp) == TERMS[-1]),
                                        perf_mode=DR,
                                    )
                                    cnt += 1
                                    if cnt % yield_every == 0:
                                        yield
                    for tsub in range(4):
                        osb = evpool.tile([P, 512], F32, tag="ev4", name="osb4")
                        nc.vector.scalar_tensor_tensor(
                            osb[:], ps4[tsub][:], INV_OW,
                            ob_sb[:, e * 512 : (e + 1) * 512], MULT, ADD,
                        )
                        nc.sync.dma_start(
                            out=out[hf * 4 + tsub, :, e * 512 : (e + 1) * 512],
                            in_=osb[:],
                        )

            def attn_slot(sl, feeder):
                n_kb = NKB[sl]
                with (
                    tc.tile_pool(name=f"mask{sl}", bufs=1) as mpool,
                    tc.tile_pool(name=f"v4{sl}", bufs=1) as v4pool,
                ):
                    msk = mpool.tile([P, 8, CH], BF16, tag="mask", name="msk")
                    nc.scalar.dma_start(
                        out=msk[:], in_=masks[sl].rearrange("m k q -> k m q")
                    )
                    for hs in range(2):
                        v4 = v4pool.tile([P, n_kb, 512], BF16, tag="v4", name="v4")
                        nc.scalar.dma_start(
                            out=v4[:, 0:4, :],
                            in_=v_gat[0][0, hs].rearrange("b p j -> p b j"),
                        )
                        nc.scalar.dma_start(
                            out=v4[:, 4:8, :],
                            in_=v_gat[0][1, hs].rearrange("b p j -> p b j"),
                        )
                        if sl == 1:
                            nc.scalar.dma_start(
                                out=v4[:, 8:12, :],
                                in_=v_gat[1][1, hs].rearrange("b p j -> p b j"),
                            )
                            nc.scalar.dma_start(
                                out=v4[:, 12:16, :],
                                in_=v_gat[1][0, hs].rearrange("b p j -> p b j"),
                            )
                        for j in range(4):
                            kv = 4 * hs + j
                            kt = kvpool.tile([P, n_kb * P], BF16, tag="kt", name="kt")
                            nc.scalar.dma_start(
                                out=kt[:, 0 : 2 * CH],
                                in_=kT_gat[0][:, kv].rearrange("c p t -> p c t"),
                            )
                            if sl == 1:
                                nc.scalar.dma_start(
                                    out=kt[:, 2 * CH : 3 * CH],
                                    in_=kT_gat[1][1, kv],
                                )
                                nc.scalar.dma_start(
                                    out=kt[:, 3 * CH : 4 * CH],
                                    in_=kT_gat[1][0, kv],
                                )
                            qt4 = qtpool.tile([P, 4, CH], BF16, tag="qt", name="qt4")
                            nc.scalar.dma_start(
                                out=qt4[:],
                                in_=qT_i[sl, kv :: KVH].rearrange("g p t -> p g t"),
                            )
                            for g in range(4):
                                h = kv + KVH * g
                                oT_ps = pspool.tile([P, CH], F32, tag="ps", name="oT_ps")
                                sums_ps = pspool.tile([P, CH], F32, tag="ps", name="sums_ps")
                                for kb in range(n_kb):
                                    st_ps = pspool.tile([P, CH], F32, tag="ps", name="st_ps")
                                    nc.tensor.matmul(
                                        st_ps[:],
                                        lhsT=kt[:, kb * P : (kb + 1) * P],
                                        rhs=qt4[:, g, :], start=True, stop=True,
                                    )
                                    pt = ptpool.tile([P, CH], BF16, tag="pt", name="pt")
                                    nc.scalar.activation(pt[:], st_ps[:], EXP, scale=SCALE)
                                    if sl == 0 or kb >= 8:
                                        mi = kb if sl == 0 else kb - 8
                                        nc.vector.tensor_mul(pt[:], pt[:], msk[:, mi, :])
                                    if kb % 2 == 0:
                                        pt_prev = pt
                                    else:
                                        pp = ptpool.tile([P, CH], BF16, tag="ptp", name="pp")
                                        nc.vector.tensor_add(pp[:], pt_prev[:], pt[:])
                                        if kb % 4 == 1:
                                            pp_prev = pp
                                        else:
                                            pq = ptpool.tile([P, CH], BF16, tag="ptq", name="pq")
                                            nc.vector.tensor_add(pq[:], pp_prev[:], pp[:])
                                            nc.tensor.matmul(
                                                sums_ps[0:1, :], lhsT=ones_col[:], rhs=pq[:],
                                                start=(kb == 3), stop=(kb == n_kb - 1),
                                            )
                                    nc.tensor.matmul(
                                        oT_ps[:],
                                        lhsT=v4[:, kb, j * P : (j + 1) * P],
                                        rhs=pt[:],
                                        start=(kb == 0), stop=(kb == n_kb - 1),
                                    )
                                    if feeder is not None:
                                        next(feeder, None)
                                rsb = rpool.tile([1, CH], F32R, tag="r", name="rsb")
                                with nc.allow_low_precision(reason="f32r softmax denom"):
                                    nc.vector.reciprocal(rsb[:], sums_ps[0:1, :])
                                rb_bc = ptpool.tile([P, CH], F32R, tag="ptr", name="rb_bc")
                                nc.gpsimd.partition_broadcast(rb_bc[:], rsb[:])
                                # o32s = SO * normalized o (ones carry 1/SO)
                                o32s = evpool.tile([P, CH], F32, tag="evb", name="o32s")
                                nc.vector.tensor_mul(o32s[:], oT_ps[:], rb_bc[:])
                                oh = evpool.tile([P, CH], FP8, tag="evh", name="oh")
                                nc.scalar.activation(oh[:], o32s[:], COPY)
                                ol = evpool.tile([P, CH], FP8, tag="evl", name="ol")
                                nc.gpsimd.tensor_sub(ol[:], o32s[:], oh[:])
                                nc.sync.dma_start(out=oT_h[sl, h], in_=oh[:])
                                nc.sync.dma_start(out=oT_l[sl, h], in_=ol[:])
                    if feeder is not None:
                        for _ in feeder:
                            pass

            # ==== P3 slot 0 woven with P0 slot 1 ====
            attn_slot(0, p0_heads(1, 4))
            p0c_cm.__exit__(None, None, None)
            w_cm.__exit__(None, None, None)
            strip_cm.__exit__(None, None, None)

            # ==== P3 slot 1 woven with P4 half 0; then P4 half 1 ====
            with tc.tile_pool(name="obp", bufs=1) as obp:
                ob_sb = obp.tile([P, D], F32, tag="ob")
                nc.sync.dma_start(
                    out=ob_sb[:], in_=ob.ap()[None, :].partition_broadcast(P)
                )
                with (
                    tc.tile_pool(name="p4a", bufs=1) as p4a,
                    tc.tile_pool(name="wb4a", bufs=4) as wb4a,
                ):
                    otr0h = p4a.tile([P, QH, CH], FP8, tag="ot0h")
                    otr0l = p4a.tile([P, QH, CH], FP8, tag="ot0l")
                    attn_slot(1, p4_half(0, 3, otr0h, otr0l, wb4a, ob_sb))
                with (
                    tc.tile_pool(name="p4b", bufs=1) as p4b,
                    tc.tile_pool(name="wb4b", bufs=4) as wb4b,
                ):
                    otr1h = p4b.tile([P, QH, CH], FP8, tag="ot1h")
                    otr1l = p4b.tile([P, QH, CH], FP8, tag="ot1l")
                    for _ in p4_half(1, 1 << 30, otr1h, otr1l, wb4b, ob_sb):
                        pass
            r_cm.__exit__(None, None, None)
            pt_cm.__exit__(None, None, None)
            qt_cm.__exit__(None, None, None)
            kv_cm.__exit__(None, None, None)

    nc.compile()
    return nc


def _get_nc():
    if "nc" not in _CACHE:
        _CACHE["nc"] = _build()
    return _CACHE["nc"]


_PERM = np.concatenate([np.arange(0, P, 2), np.arange(1, P, 2)])


def _split8(a, s):
    """Return (hi, lo) fp8 arrays of a*s."""
    a = np.clip(a * s, -240.0, 240.0)
    hi = a.astype(E4)
    lo = (a - hi.astype(np.float32)).astype(E4)
    return hi, lo


def _prep_shared(qw_w, qw_b, kw_w, kw_b, vw_w, vw_b, ow_w, ow_b, fc, fs):
    f32 = np.float32
    c = np.ascontiguousarray

    # [h, dp, dt, fp] = w[h*128 + perm[fp], dt*128 + dp]
    qq = qw_w.reshape(QH, P, D)[:, _PERM, :]                      # [h, fp, d]
    qwT = qq.reshape(QH, P, ND, P).transpose(0, 3, 2, 1)
    qwT8 = np.stack(_split8(qwT, SW))                             # [2, h, dp, dt, fp]
    kk = kw_w.reshape(KVH, P, D)[:, _PERM, :]
    kwT = kk.reshape(KVH, P, ND, P).transpose(0, 3, 2, 1)
    kwT8 = np.stack(_split8(kwT, SW))
    # [hs, kp, dp, 2, j] = vw[hs*512 + j, (2kp+two)*128 + dp]
    vwT = vw_w.reshape(2, 512, NKP, 2, P).transpose(0, 2, 4, 3, 1)
    vwT8 = np.stack(_split8(vwT, SW))
    # [e, fpair, fp, 2, j] = ow[e*512 + j, (2fpair+two)*128 + fp]
    owT = ow_w.reshape(8, 512, NKP, 2, P).transpose(0, 2, 4, 3, 1)
    owT8 = np.stack(_split8(owT, SW))
    cos_all = c(fc.T.astype(f32)) * np.float32(INV_XW)  # [64, S], carries de-scale
    sin_all = c(fs.T.astype(f32)) * np.float32(INV_XW)
    qbT = c(qw_b.reshape(QH, P)[:, _PERM].T.astype(f32)) * np.float32(SX * SW)
    kbT = c(kw_b.reshape(KVH, P)[:, _PERM].T.astype(f32)) * np.float32(SX * SW)
    return dict(
        qwT8=c(qwT8), kwT8=c(kwT8), vwT8=c(vwT8), owT8=c(owT8),
        cos_all=cos_all, sin_all=sin_all, qbT=qbT, kbT=kbT,
        vb=c(vw_b.astype(f32)), ob=c(ow_b.astype(f32)),
    )


def _masks_for(chunks):
    m = np.zeros((2, 8, P, CH), BF)
    kp = np.arange(P)[:, None]
    qi = np.arange(CH)[None, :]
    for sl in range(2):
        q0 = chunks[sl] * CH
        for mi in range(8):
            kb = mi if sl == 0 else mi + 8
            m[sl, mi] = (kb * P + kp <= q0 + qi).astype(BF)
    return m


def _core_chunks(core):
    b, par = core // 2, core % 2
    return b, ((0, 3) if par == 0 else (1, 2))


def _make_in_maps(inputs):
    """inputs: dict with the reference's setup_inputs() keys (numpy)."""
    g = lambda k: np.asarray(inputs[k])
    shared = _prep_shared(
        g("qw_w"), g("qw_b"), g("kw_w"), g("kw_b"), g("vw_w"), g("vw_b"),
        g("ow_w"), g("ow_b"), g("freqs_cos"), g("freqs_sin"),
    )
    input = g("input")
    in_maps = []
    for core in range(NCORES):
        b, chunks = _core_chunks(core)
        x = input[b].astype(np.float32)  # [S, D]
        # [s, dp, dt, t] = x[s*512 + t, dt*128 + dp]
        strips = x.reshape(NCH, CH, ND, P).transpose(0, 3, 2, 1)
        own_f = strips[list(chunks)]                       # [2, dp, dt, t]
        own8 = np.stack(_split8(own_f, SX))                # [2(hl), 2, dp, dt, t]
        cos_own = np.ascontiguousarray(
            np.stack([shared["cos_all"][:, c * CH : (c + 1) * CH] for c in chunks], 1)
        )
        sin_own = np.ascontiguousarray(
            np.stack([shared["sin_all"][:, c * CH : (c + 1) * CH] for c in chunks], 1)
        )
        m = {k: v for k, v in shared.items() if k not in ("cos_all", "sin_all")}
        m.update(
            ones=np.full(P, 1.0 / SO, BF),
            own8=np.ascontiguousarray(own8),
            cos_own=cos_own, sin_own=sin_own, masks=_masks_for(chunks),
        )
        in_maps.append(m)
    return in_maps


def kernel(input, freqs_cos, freqs_sin, qw_w, qw_b, kw_w, kw_b, vw_w, vw_b,
           ow_w, ow_b, start_pos):
    in_maps = _make_in_maps(dict(
        input=input, freqs_cos=freqs_cos, freqs_sin=freqs_sin,
        qw_w=qw_w, qw_b=qw_b, kw_w=kw_w, kw_b=kw_b, vw_w=vw_w, vw_b=vw_b,
        ow_w=ow_w, ow_b=ow_b,
    ))
    nc = _get_nc()
    res = run_bass_kernel_spmd(nc, in_maps, list(range(NCORES)))

    out = np.empty((B, S, D), np.float32)
    for core in range(NCORES):
        b, chunks = _core_chunks(core)
        r = res.results[core]["out"].reshape(2, CH, D)
        for sl in range(2):
            c0 = chunks[sl] * CH
            out[b, c0 : c0 + CH, :] = r[sl]
    return out


# revision 21
# speedup vs baseline: 1.0507x; 1.0148x over previous
"""GQA attention prefill (B=4, S=2048, D=4096, 32 q-heads / 8 kv-heads, rotary,
causal) on 8 TRN2 NeuronCores.

Sharding: token-parallel ("zigzag" sequence split) — core c handles batch
c//2 and two 512-token chunks of its sequence: chunks {0,3} for even cores,
{1,2} for odd cores (balances the causal triangle). Each core computes Q and
K/V projections for ITS OWN tokens only; the K/V halves are exchanged with
the partner core (same batch) via two pairwise AllGather collectives, so the
K/V projection work is not duplicated. Attention and the output projection
run on each core's own tokens; outputs are disjoint token slices gathered on
host.

Precision: all four projections (Q/K/V/O) run as 3-term split-fp8 GEMMs in
DoubleRow perf mode (4x PE rate): x ~ (xh + xl), w ~ (wh + wl) with
power-of-2 pre-scales chosen to keep fp8e4m3 values in the normal range;
x@w ~ xh@wh + xh@wl + xl@wh accumulated in one PSUM group. The de-scale
folds into the eviction constants (rotary cos/sin tables and bias vectors)
at zero device cost. Attention scores/AV run f32r/bf16 exactly as before.

Device layout conventions:
  - activations for QK^T are kept transposed: [head_dim (partitions), tokens]
  - rotary pairs are de-interleaved (even dims -> partitions 0-63, odd ->
    64-127) via a host-side permutation of the qw/kw rows; scores are
    invariant to this shared permutation.
  - attention runs in "scores-transposed" orientation: ST[key, query] =
    kT.T @ qT, softmax over the partition (key) axis with the denominator
    computed by a ones-vector matmul; no max-subtraction (scores are O(1)).
  - DMA routing: large batched loads on nc.sync (HWDGE); high-count
    weight/output streams on nc.gpsimd (SWDGE, Pool engine).
"""

import numpy as np
import ml_dtypes

import concourse.bacc as bacc
import concourse.bass as bass
import concourse.tile as tile
from concourse import library_config, mybir
from concourse.bass_utils import run_bass_kernel_spmd

F32 = mybir.dt.float32
F32R = mybir.dt.float32r
BF16 = mybir.dt.bfloat16
FP8 = mybir.dt.float8e4
DR = mybir.MatmulPerfMode.DoubleRow
EXP = mybir.ActivationFunctionType.Exp
COPY = mybir.ActivationFunctionType.Copy
ADD = mybir.AluOpType.add
MULT = mybir.AluOpType.mult
SUB = mybir.AluOpType.subtract

B, S, D = 4, 2048, 4096
QH, KVH, HEAD = 32, 8, 128
P = 128
CH = 512                # token chunk (= query tile)
NCH = S // CH           # 4 chunks per sequence
ND = D // P             # 32 d-tiles
NKP = ND // 2           # 16 DoubleRow k-tile pairs
NCORES = 8
NKB = (8, 16)           # key-blocks per query slot (padded, uniform)
SCALE = 1.0 / np.sqrt(HEAD)
BF = ml_dtypes.bfloat16
E4 = ml_dtypes.float8_e4m3

SX = 32.0               # fp8 pre-scale: activations (input x)
SW = 1024.0             # fp8 pre-scale: projection weights
SO = 32.0               # fp8 pre-scale: attention output (o_attn)
INV_XW = 1.0 / (SX * SW)
INV_OW = 1.0 / (SO * SW)

_CACHE = {}


def _build():
    nc = bacc.Bacc("TRN2", target_bir_lowering=False, debug=False, num_devices=NCORES)

    # ---- per-core external inputs ----
    # own strips, hi/lo fp8: [hl, slot, dp, dt, t]
    own = nc.dram_tensor("own8", [2, 2, P, ND, CH], FP8, kind="ExternalInput")
    # weights, hi/lo fp8; [hl, head, dp, dt, fp] for q/k
    qwT = nc.dram_tensor("qwT8", [2, QH, P, ND, P], FP8, kind="ExternalInput")
    kwT = nc.dram_tensor("kwT8", [2, KVH, P, ND, P], FP8, kind="ExternalInput")
    # [hl, hs, kp, dp, 2, j]
    vwT = nc.dram_tensor("vwT8", [2, 2, NKP, P, 2, 512], FP8, kind="ExternalInput")
    # [hl, e, fpair, fp, 2, j]
    owT = nc.dram_tensor("owT8", [2, 8, NKP, P, 2, 512], FP8, kind="ExternalInput")
    cos_own = nc.dram_tensor("cos_own", [64, 2, CH], F32, kind="ExternalInput")
    sin_own = nc.dram_tensor("sin_own", [64, 2, CH], F32, kind="ExternalInput")
    qbT = nc.dram_tensor("qbT", [P, QH], F32, kind="ExternalInput")
    kbT = nc.dram_tensor("kbT", [P, KVH], F32, kind="ExternalInput")
    vb = nc.dram_tensor("vb", [KVH * HEAD], F32, kind="ExternalInput")
    ob = nc.dram_tensor("ob", [D], F32, kind="ExternalInput")
    masks = nc.dram_tensor("masks", [2, 8, P, CH], BF16, kind="ExternalInput")
    ones = nc.dram_tensor("ones", [P], BF16, kind="ExternalInput")

    # ---- internal DRAM ----
    qT_i = nc.dram_tensor("qT_i", [2, QH, P, CH], BF16)
    # own K/V halves + pairwise-gathered full tensors
    # per-slot tensors (separate handles so the slot-0 collective's read does
    # not create a false whole-tensor WAR dependency against slot-1 writes)
    kT_own = [nc.dram_tensor(f"kT_own{s}", [KVH, P, CH], BF16) for s in range(2)]
    kT_gat = [nc.dram_tensor(f"kT_gat{s}", [2, KVH, P, CH], BF16) for s in range(2)]
    v_own = [nc.dram_tensor(f"v_own{s}", [2, 4, P, 512], BF16) for s in range(2)]
    v_gat = [nc.dram_tensor(f"v_gat{s}", [2, 2, 4, P, 512], BF16) for s in range(2)]
    oT_h = nc.dram_tensor("oT_h", [2, QH, P, CH], FP8)
    oT_l = nc.dram_tensor("oT_l", [2, QH, P, CH], FP8)

    out = nc.dram_tensor("out", [8, P, D], F32, kind="ExternalOutput")

    PAIRS = [[0, 1], [2, 3], [4, 5], [6, 7]]

    with tile.TileContext(nc) as tc:
        nc.gpsimd.load_library(library_config.lib)
        with (
            tc.tile_pool(name="const", bufs=1) as const,
            tc.tile_pool(name="ev", bufs=4) as evpool,
            tc.tile_pool(name="rt", bufs=4) as rtpool,
            tc.tile_pool(name="ps", bufs=8, space="PSUM") as pspool,
        ):
            kbT_e = const.tile([64, KVH], F32, tag="kbte")
            kbT_o = const.tile([64, KVH], F32, tag="kbto")
            nc.sync.dma_start(out=kbT_e[:], in_=kbT[0:64, :])
            nc.sync.dma_start(out=kbT_o[:], in_=kbT[64:P, :])
            ones_col = const.tile([P, 1], BF16, tag="oc")
            nc.sync.dma_start(out=ones_col[:], in_=ones.ap()[:, None])
            cos_own_sb = const.tile([64, 2, CH], F32, tag="coso")
            sin_own_sb = const.tile([64, 2, CH], F32, tag="sino")
            nc.sync.dma_start(out=cos_own_sb[:], in_=cos_own[:])
            nc.sync.dma_start(out=sin_own_sb[:], in_=sin_own[:])

            def rotary_evict(ps, dst, cos_ap, sin_ap, be, bo):
                """dst[0:64]=(pe+be)*cos-(po+bo)*sin; dst[64:128]=(pe+be)*sin+(po+bo)*cos

                cos/sin tables carry the 1/(SX*SW) de-scale; be/bo carry SX*SW.
                """
                pe, po = ps[0:64, :], ps[64:128, :]
                t1 = rtpool.tile([64, CH], F32, tag="rt", name="t1")
                t2 = rtpool.tile([64, CH], F32, tag="rt", name="t2")
                nc.vector.scalar_tensor_tensor(t1[:], pe, be, cos_ap, ADD, MULT)
                nc.vector.scalar_tensor_tensor(t2[:], po, bo, sin_ap, ADD, MULT)
                nc.vector.tensor_sub(dst[0:64, :], t1[:], t2[:])
                t3 = rtpool.tile([64, CH], F32, tag="rt", name="t3")
                t4 = rtpool.tile([64, CH], F32, tag="rt", name="t4")
                nc.vector.scalar_tensor_tensor(t3[:], pe, be, sin_ap, ADD, MULT)
                nc.vector.scalar_tensor_tensor(t4[:], po, bo, cos_ap, ADD, MULT)
                nc.vector.tensor_add(dst[64:128, :], t3[:], t4[:])

            # attention-phase pools entered first so later pools (strip/w)
            # can release in LIFO order right after attention slot 0
            kv_cm = tc.tile_pool(name="kvS", bufs=3)
            kvpool = kv_cm.__enter__()
            qt_cm = tc.tile_pool(name="qtS", bufs=2)
            qtpool = qt_cm.__enter__()
            pt_cm = tc.tile_pool(name="ptS", bufs=4)
            ptpool = pt_cm.__enter__()
            r_cm = tc.tile_pool(name="rS", bufs=2)
            rpool = r_cm.__enter__()

            # ---- own strips (hi/lo fp8), used by K/V and Q projections ----
            strip_cm = tc.tile_pool(name="strip", bufs=1)
            strip_pool = strip_cm.__enter__()
            st = {}
            for sl in range(2):
                for hl in range(2):
                    t = strip_pool.tile([P, ND, CH], FP8, tag=f"strip{hl}{sl}",
                                        name=f"st{hl}_{sl}")
                    nc.sync.dma_start(out=t[:, 0:8, :], in_=own[hl, sl, :, 0:8])
                    nc.sync.dma_start(out=t[:, 8:ND, :], in_=own[hl, sl, :, 8:ND])
                    st[(hl, sl)] = t

            w_cm = tc.tile_pool(name="w", bufs=8)
            wpool = w_cm.__enter__()

            # 3-term schedule: (x_part, w_part) pairs, ll dropped
            TERMS = [(0, 0), (0, 1), (1, 0)]

            # ============ P1a: K projection over own chunks ============
            vb_cm = tc.tile_pool(name="p1c", bufs=1)
            p1c = vb_cm.__enter__()
            vb_sb = p1c.tile([P, KVH * HEAD], F32, tag="vb")
            nc.sync.dma_start(
                out=vb_sb[:], in_=vb.ap()[None, :].partition_broadcast(P)
            )
            for sl in range(2):
                for kv in range(KVH):
                    wk = []
                    for hl in range(2):
                        w = wpool.tile([P, ND, P], FP8, tag="w",
                                       name=f"kw{sl}_{kv}_{hl}")
                        nc.gpsimd.dma_start(out=w[:], in_=kwT[hl, kv])
                        wk.append(w)
                    ps = pspool.tile([P, CH], F32, tag="ps", name="ps_k")
                    n3 = len(TERMS) * NKP
                    i = 0
                    for xp, wp in TERMS:
                        for kp in range(NKP):
                            nc.tensor.matmul(
                                ps[:],
                                lhsT=wk[wp][:, 2 * kp : 2 * kp + 2, :],
                                rhs=st[(xp, sl)][:, 2 * kp : 2 * kp + 2, :],
                                start=(i == 0), stop=(i == n3 - 1),
                                perf_mode=DR,
                            )
                            i += 1
                    krot = evpool.tile([P, CH], BF16, tag="ev", bufs=8, name="krot")
                    rotary_evict(
                        ps, krot,
                        cos_own_sb[:, sl, :], sin_own_sb[:, sl, :],
                        kbT_e[:, kv : kv + 1], kbT_o[:, kv : kv + 1],
                    )
                    nc.sync.dma_start(out=kT_own[sl][kv], in_=krot[:])
                if sl == 0:
                    # pairwise K exchange for slot-0 chunks: lands well
                    # before attention slot 0 needs it
                    nc.gpsimd.collective_compute(
                        "AllGather", mybir.AluOpType.bypass, replica_groups=PAIRS,
                        ins=[kT_own[0][:]], outs=[kT_gat[0][:]],
                    )

            # ============ P1b: V projection over own chunks ============
            with tc.tile_pool(name="wb", bufs=6) as wbpool:
                for sl in range(2):
                    for hs in range(2):
                        psv = [
                            pspool.tile([P, 512], F32, tag="ps", name=f"psv{i}")
                            for i in range(4)
                        ]
                        for kp in range(NKP):
                            vw = []
                            for hl in range(2):
                                w = wbpool.tile([P, 2, 512], FP8, tag="wb",
                                                name=f"vw{hl}")
                                # sync queue: keeps these off the Pool SEQ
                                # (collective input waits) and off Act (the
                                # scheduler hoists collective-dependent attn
                                # loads there, which would stall these)
                                nc.sync.dma_start(out=w[:], in_=vwT[hl, hs, kp])
                                vw.append(w)
                            for xp, wp in TERMS:
                                for tt in range(4):
                                    nc.tensor.matmul(
                                        psv[tt][:],
                                        lhsT=st[(xp, sl)][
                                            :, 2 * kp : 2 * kp + 2,
                                            tt * P : (tt + 1) * P,
                                        ],
                                        rhs=vw[wp][:],
                                        start=(kp == 0 and (xp, wp) == TERMS[0]),
                                        stop=(kp == NKP - 1 and (xp, wp) == TERMS[-1]),
                                        perf_mode=DR,
                                    )
                        for tt in range(4):
                            vsb = evpool.tile([P, 512], BF16, tag="evb", name="vsb")
                            nc.vector.scalar_tensor_tensor(
                                vsb[:], psv[tt][:], INV_XW,
                                vb_sb[:, hs * 512 : (hs + 1) * 512], MULT, ADD,
                            )
                            nc.sync.dma_start(out=v_own[sl][hs, tt], in_=vsb[:])
                    # pairwise V exchange for this slot's chunks; the
                    # slot-1 K exchange is sequenced after the slot-0 V one
                    # (attention needs V0 ~500us before K1)
                    nc.gpsimd.collective_compute(
                        "AllGather", mybir.AluOpType.bypass, replica_groups=PAIRS,
                        ins=[v_own[sl][:]], outs=[v_gat[sl][:]],
                    )
                    if sl == 0:
                        nc.gpsimd.collective_compute(
                            "AllGather", mybir.AluOpType.bypass,
                            replica_groups=PAIRS,
                            ins=[kT_own[1][:]], outs=[kT_gat[1][:]],
                        )

            vb_cm.__exit__(None, None, None)

            # ============ P0: Q projection + rotary -> qT_i ============
            # Slot 0 is emitted eagerly; slot 1 is a generator woven into
            # P3-slot0's key-block loop.
            p0c_cm = tc.tile_pool(name="p0c", bufs=1)
            p0c = p0c_cm.__enter__()
            qbT_e = p0c.tile([64, QH], F32, tag="qbte")
            qbT_o = p0c.tile([64, QH], F32, tag="qbto")
            nc.sync.dma_start(out=qbT_e[:], in_=qbT[0:64, :])
            nc.sync.dma_start(out=qbT_o[:], in_=qbT[64:P, :])

            def p0_heads(sl, group):
                """Emit Q-proj for one slot; yield after each `group` matmuls."""
                for h in range(QH):
                    wq = []
                    for hl in range(2):
                        w = wpool.tile([P, ND, P], FP8, tag="w",
                                       name=f"qw{sl}_{h}_{hl}")
                        eng = nc.gpsimd if hl == 0 else nc.sync
                        eng.dma_start(out=w[:], in_=qwT[hl, h])
                        wq.append(w)
                    ps = pspool.tile([P, CH], F32, tag="ps", name="ps_q")
                    n3 = len(TERMS) * NKP
                    i = 0
                    for xp, wp in TERMS:
                        for kp in range(NKP):
                            nc.tensor.matmul(
                                ps[:],
                                lhsT=wq[wp][:, 2 * kp : 2 * kp + 2, :],
                                rhs=st[(xp, sl)][:, 2 * kp : 2 * kp + 2, :],
                                start=(i == 0), stop=(i == n3 - 1),
                                perf_mode=DR,
                            )
                            i += 1
                            if i % group == 0:
                                yield
                    qrot = evpool.tile([P, CH], BF16, tag="ev", bufs=8, name="qrot")
                    rotary_evict(
                        ps, qrot,
                        cos_own_sb[:, sl, :], sin_own_sb[:, sl, :],
                        qbT_e[:, h : h + 1], qbT_o[:, h : h + 1],
                    )
                    nc.sync.dma_start(out=qT_i[sl, h], in_=qrot[:])

            for _ in p0_heads(0, ND):
                pass


            def p4_half(hf, yield_every, otr_h, otr_l, wb4pool, ob_sb):
                """Emit o-proj for token-slot half `hf` (ttiles 4hf..4hf+3)."""
                for hq in range(0, QH, 8):
                    nc.sync.dma_start(
                        out=otr_h[:, hq : hq + 8, :],
                        in_=oT_h[hf, hq : hq + 8].rearrange("h p t -> p h t"),
                    )
                    nc.sync.dma_start(
                        out=otr_l[:, hq : hq + 8, :],
                        in_=oT_l[hf, hq : hq + 8].rearrange("h p t -> p h t"),
                    )
                otr = (otr_h, otr_l)
                for e in range(8):
                    ps4 = [
                        pspool.tile([P, 512], F32, tag="ps", name=f"ps4_{i}")
                        for i in range(4)
                    ]
                    cnt = 0
                    n3 = len(TERMS) * NKP
                    for fq in range(NKP // 4):
                        ow = []
                        for hl in range(2):
                            w = wb4pool.tile([P, 4, 2, 512], FP8, tag="wb4",
                                             name=f"ow{hl}")
                            nc.gpsimd.dma_start(
                                out=w[:],
                                in_=owT[hl, e, 4 * fq : 4 * fq + 4].rearrange(
                                    "q p two j -> p q two j"
                                ),
                            )
                            ow.append(w)
                        for df in range(4):
                            fp = 4 * fq + df
                            for xp, wp in TERMS:
                                for tsub in range(4):
                                    nc.tensor.matmul(
                                        ps4[tsub][:],
                                        lhsT=otr[xp][
                                            :, 2 * fp : 2 * fp + 2,
                                            tsub * P : (tsub + 1) * P,
                                        ],
                                        rhs=ow[wp][:, df, :, :],
                                        start=(fp == 0 and (xp, wp) == TERMS[0]),
                                        stop=(fp == NKP - 1
                                              and (xp, wp) == TERMS[-1]),
                                        perf_mode=DR,
                                    )
                                    cnt += 1
                                    if cnt % yield_every == 0:
                                        yield
                    for tsub in range(4):
                        osb = evpool.tile([P, 512], F32, tag="ev4", name="osb4")
                        nc.vector.scalar_tensor_tensor(
                            osb[:], ps4[tsub][:], INV_OW,
                            ob_sb[:, e * 512 : (e + 1) * 512], MULT, ADD,
                        )
                        nc.sync.dma_start(
                            out=out[hf * 4 + tsub, :, e * 512 : (e + 1) * 512],
                            in_=osb[:],
                        )

            def attn_slot(sl, feeder):
                n_kb = NKB[sl]
                # logical-time floor: stops the scheduler hoisting these
                # collective-dependent loads into the projection phase, where
                # their completion-semaphore epochs stall unrelated DMAs
                load_floor_ms = 0.25 if sl == 0 else 0.45
                with (
                    tc.tile_pool(name=f"mask{sl}", bufs=1) as mpool,
                    tc.tile_pool(name=f"v4{sl}", bufs=1) as v4pool,
                ):
                    with tc.tile_wait_until(load_floor_ms):
                        msk = mpool.tile([P, 8, CH], BF16, tag="mask", name="msk")
                        nc.scalar.dma_start(
                            out=msk[:], in_=masks[sl].rearrange("m k q -> k m q")
                        )
                    for hs in range(2):
                        with tc.tile_wait_until(load_floor_ms):
                            v4 = v4pool.tile([P, n_kb, 512], BF16, tag="v4", name="v4")
                            nc.scalar.dma_start(
                                out=v4[:, 0:4, :],
                                in_=v_gat[0][0, hs].rearrange("b p j -> p b j"),
                            )
                            nc.scalar.dma_start(
                                out=v4[:, 4:8, :],
                                in_=v_gat[0][1, hs].rearrange("b p j -> p b j"),
                            )
                            if sl == 1:
                                nc.scalar.dma_start(
                                    out=v4[:, 8:12, :],
                                    in_=v_gat[1][1, hs].rearrange("b p j -> p b j"),
                                )
                                nc.scalar.dma_start(
                                    out=v4[:, 12:16, :],
                                    in_=v_gat[1][0, hs].rearrange("b p j -> p b j"),
                                )
                        for j in range(4):
                            kv = 4 * hs + j
                            with tc.tile_wait_until(load_floor_ms):
                                kt = kvpool.tile([P, n_kb * P], BF16, tag="kt", name="kt")
                                nc.scalar.dma_start(
                                    out=kt[:, 0 : 2 * CH],
                                    in_=kT_gat[0][:, kv].rearrange("c p t -> p c t"),
                                )
                                if sl == 1:
                                    nc.scalar.dma_start(
                                        out=kt[:, 2 * CH : 3 * CH],
                                        in_=kT_gat[1][1, kv],
                                    )
                                    nc.scalar.dma_start(
                                        out=kt[:, 3 * CH : 4 * CH],
                                        in_=kT_gat[1][0, kv],
                                    )
                                qt4 = qtpool.tile([P, 4, CH], BF16, tag="qt", name="qt4")
                                nc.scalar.dma_start(
                                    out=qt4[:],
                                    in_=qT_i[sl, kv :: KVH].rearrange("g p t -> p g t"),
                                )
                            for g in range(4):
                                h = kv + KVH * g
                                oT_ps = pspool.tile([P, CH], F32, tag="ps", name="oT_ps")
                                sums_ps = pspool.tile([P, CH], F32, tag="ps", name="sums_ps")
                                for kb in range(n_kb):
                                    st_ps = pspool.tile([P, CH], F32, tag="ps", name="st_ps")
                                    nc.tensor.matmul(
                                        st_ps[:],
                                        lhsT=kt[:, kb * P : (kb + 1) * P],
                                        rhs=qt4[:, g, :], start=True, stop=True,
                                    )
                                    pt = ptpool.tile([P, CH], BF16, tag="pt", name="pt")
                                    nc.scalar.activation(pt[:], st_ps[:], EXP, scale=SCALE)
                                    if sl == 0 or kb >= 8:
                                        mi = kb if sl == 0 else kb - 8
                                        nc.vector.tensor_mul(pt[:], pt[:], msk[:, mi, :])
                                    if kb % 2 == 0:
                                        pt_prev = pt
                                    else:
                                        pp = ptpool.tile([P, CH], BF16, tag="ptp", name="pp")
                                        nc.vector.tensor_add(pp[:], pt_prev[:], pt[:])
                                        if kb % 4 == 1:
                                            pp_prev = pp
                                        else:
                                            pq = ptpool.tile([P, CH], BF16, tag="ptq", name="pq")
                                            nc.vector.tensor_add(pq[:], pp_prev[:], pp[:])
                                            nc.tensor.matmul(
                                                sums_ps[0:1, :], lhsT=ones_col[:], rhs=pq[:],
                                                start=(kb == 3), stop=(kb == n_kb - 1),
                                            )
                                    nc.tensor.matmul(
                                        oT_ps[:],
                                        lhsT=v4[:, kb, j * P : (j + 1) * P],
                                        rhs=pt[:],
                                        start=(kb == 0), stop=(kb == n_kb - 1),
                                    )
                                    if feeder is not None:
                                        next(feeder, None)
                                rsb = rpool.tile([1, CH], F32R, tag="r", name="rsb")
                                with nc.allow_low_precision(reason="f32r softmax denom"):
                                    nc.vector.reciprocal(rsb[:], sums_ps[0:1, :])
                                rb_bc = ptpool.tile([P, CH], F32R, tag="ptr", name="rb_bc")
                                nc.gpsimd.partition_broadcast(rb_bc[:], rsb[:])
                                # o32s = SO * normalized o (ones carry 1/SO)
                                o32s = evpool.tile([P, CH], F32, tag="evb", name="o32s")
                                nc.vector.tensor_mul(o32s[:], oT_ps[:], rb_bc[:])
                                oh = evpool.tile([P, CH], FP8, tag="evh", name="oh")
                                nc.scalar.activation(oh[:], o32s[:], COPY)
                                ol = evpool.tile([P, CH], FP8, tag="evl", name="ol")
                                nc.gpsimd.tensor_sub(ol[:], o32s[:], oh[:])
                                nc.sync.dma_start(out=oT_h[sl, h], in_=oh[:])
                                nc.sync.dma_start(out=oT_l[sl, h], in_=ol[:])
                    if feeder is not None:
                        for _ in feeder:
                            pass

            # ==== P3 slot 0 woven with P0 slot 1 ====
            attn_slot(0, p0_heads(1, 4))
            p0c_cm.__exit__(None, None, None)
            w_cm.__exit__(None, None, None)
            strip_cm.__exit__(None, None, None)

            # ==== P3 slot 1 woven with P4 half 0; then P4 half 1 ====
            with tc.tile_pool(name="obp", bufs=1) as obp:
                ob_sb = obp.tile([P, D], F32, tag="ob")
                nc.sync.dma_start(
                    out=ob_sb[:], in_=ob.ap()[None, :].partition_broadcast(P)
                )
                with (
                    tc.tile_pool(name="p4a", bufs=1) as p4a,
                    tc.tile_pool(name="wb4a", bufs=4) as wb4a,
                ):
                    otr0h = p4a.tile([P, QH, CH], FP8, tag="ot0h")
                    otr0l = p4a.tile([P, QH, CH], FP8, tag="ot0l")
                    attn_slot(1, p4_half(0, 3, otr0h, otr0l, wb4a, ob_sb))
                with (
                    tc.tile_pool(name="p4b", bufs=1) as p4b,
                    tc.tile_pool(name="wb4b", bufs=4) as wb4b,
                ):
                    otr1h = p4b.tile([P, QH, CH], FP8, tag="ot1h")
                    otr1l = p4b.tile([P, QH, CH], FP8, tag="ot1l")
                    for _ in p4_half(1, 1 << 30, otr1h, otr1l, wb4b, ob_sb):
                        pass
            r_cm.__exit__(None, None, None)
            pt_cm.__exit__(None, None, None)
            qt_cm.__exit__(None, None, None)
            kv_cm.__exit__(None, None, None)

    nc.compile()
    return nc


def _get_nc():
    if "nc" not in _CACHE:
        _CACHE["nc"] = _build()
    return _CACHE["nc"]


_PERM = np.concatenate([np.arange(0, P, 2), np.arange(1, P, 2)])


def _split8(a, s):
    """Return (hi, lo) fp8 arrays of a*s."""
    a = np.clip(a * s, -240.0, 240.0)
    hi = a.astype(E4)
    lo = (a - hi.astype(np.float32)).astype(E4)
    return hi, lo


def _prep_shared(qw_w, qw_b, kw_w, kw_b, vw_w, vw_b, ow_w, ow_b, fc, fs):
    f32 = np.float32
    c = np.ascontiguousarray

    # [h, dp, dt, fp] = w[h*128 + perm[fp], dt*128 + dp]
    qq = qw_w.reshape(QH, P, D)[:, _PERM, :]                      # [h, fp, d]
    qwT = qq.reshape(QH, P, ND, P).transpose(0, 3, 2, 1)
    qwT8 = np.stack(_split8(qwT, SW))                             # [2, h, dp, dt, fp]
    kk = kw_w.reshape(KVH, P, D)[:, _PERM, :]
    kwT = kk.reshape(KVH, P, ND, P).transpose(0, 3, 2, 1)
    kwT8 = np.stack(_split8(kwT, SW))
    # [hs, kp, dp, 2, j] = vw[hs*512 + j, (2kp+two)*128 + dp]
    vwT = vw_w.reshape(2, 512, NKP, 2, P).transpose(0, 2, 4, 3, 1)
    vwT8 = np.stack(_split8(vwT, SW))
    # [e, fpair, fp, 2, j] = ow[e*512 + j, (2fpair+two)*128 + fp]
    owT = ow_w.reshape(8, 512, NKP, 2, P).transpose(0, 2, 4, 3, 1)
    owT8 = np.stack(_split8(owT, SW))
    cos_all = c(fc.T.astype(f32)) * np.float32(INV_XW)  # [64, S], carries de-scale
    sin_all = c(fs.T.astype(f32)) * np.float32(INV_XW)
    qbT = c(qw_b.reshape(QH, P)[:, _PERM].T.astype(f32)) * np.float32(SX * SW)
    kbT = c(kw_b.reshape(KVH, P)[:, _PERM].T.astype(f32)) * np.float32(SX * SW)
    return dict(
        qwT8=c(qwT8), kwT8=c(kwT8), vwT8=c(vwT8), owT8=c(owT8),
        cos_all=cos_all, sin_all=sin_all, qbT=qbT, kbT=kbT,
        vb=c(vw_b.astype(f32)), ob=c(ow_b.astype(f32)),
    )


def _masks_for(chunks):
    m = np.zeros((2, 8, P, CH), BF)
    kp = np.arange(P)[:, None]
    qi = np.arange(CH)[None, :]
    for sl in range(2):
        q0 = chunks[sl] * CH
        for mi in range(8):
            kb = mi if sl == 0 else mi + 8
            m[sl, mi] = (kb * P + kp <= q0 + qi).astype(BF)
    return m


def _core_chunks(core):
    b, par = core // 2, core % 2
    return b, ((0, 3) if par == 0 else (1, 2))


def _make_in_maps(inputs):
    """inputs: dict with the reference's setup_inputs() keys (numpy)."""
    g = lambda k: np.asarray(inputs[k])
    shared = _prep_shared(
        g("qw_w"), g("qw_b"), g("kw_w"), g("kw_b"), g("vw_w"), g("vw_b"),
        g("ow_w"), g("ow_b"), g("freqs_cos"), g("freqs_sin"),
    )
    input = g("input")
    in_maps = []
    for core in range(NCORES):
        b, chunks = _core_chunks(core)
        x = input[b].astype(np.float32)  # [S, D]
        # [s, dp, dt, t] = x[s*512 + t, dt*128 + dp]
        strips = x.reshape(NCH, CH, ND, P).transpose(0, 3, 2, 1)
        own_f = strips[list(chunks)]                       # [2, dp, dt, t]
        own8 = np.stack(_split8(own_f, SX))                # [2(hl), 2, dp, dt, t]
        cos_own = np.ascontiguousarray(
            np.stack([shared["cos_all"][:, c * CH : (c + 1) * CH] for c in chunks], 1)
        )
        sin_own = np.ascontiguousarray(
            np.stack([shared["sin_all"][:, c * CH : (c + 1) * CH] for c in chunks], 1)
        )
        m = {k: v for k, v in shared.items() if k not in ("cos_all", "sin_all")}
        m.update(
            ones=np.full(P, 1.0 / SO, BF),
            own8=np.ascontiguousarray(own8),
            cos_own=cos_own, sin_own=sin_own, masks=_masks_for(chunks),
        )
        in_maps.append(m)
    return in_maps


def kernel(input, freqs_cos, freqs_sin, qw_w, qw_b, kw_w, kw_b, vw_w, vw_b,
           ow_w, ow_b, start_pos):
    in_maps = _make_in_maps(dict(
        input=input, freqs_cos=freqs_cos, freqs_sin=freqs_sin,
        qw_w=qw_w, qw_b=qw_b, kw_w=kw_w, kw_b=kw_b, vw_w=vw_w, vw_b=vw_b,
        ow_w=ow_w, ow_b=ow_b,
    ))
    nc = _get_nc()
    res = run_bass_kernel_spmd(nc, in_maps, list(range(NCORES)))

    out = np.empty((B, S, D), np.float32)
    for core in range(NCORES):
        b, chunks = _core_chunks(core)
        r = res.results[core]["out"].reshape(2, CH, D)
        for sl in range(2):
            c0 = chunks[sl] * CH
            out[b, c0 : c0 + CH, :] = r[sl]
    return out
